# revision 18
# baseline (speedup 1.0000x reference)
"""Bass/Trainium2 kernel for nn_AlexNetOWT_BN (binarized AlexNet, batch 64).

Strategy
--------
Data-parallel convolutions (8 images per core, conv weights replicated as
sign() values in fp8e4 -> exact integer arithmetic in fp32 PSUM), conv1 in a
hi/lo integer-split fp16 pair (exact PE arithmetic, ~3e-6 total error which is
far inside the min |bn-output| margin of ~2.4e-5), conv2 in fp8 DoubleRow
(K=256 per matmul: both ci tiles fused, halving PE stream cycles),
tensor-parallel FC layers (columns sharded 8-ways, activations exchanged via
AllGather), log_softmax computed redundantly on every core; core 0's [64,1000]
output is returned.

Conv biases and FC biases are folded into the BN shift on the host
(sign(inv*(x+cb)+shf) == sign(inv*x + (inv*cb+shf)), and max-pool commutes
with the per-channel constant add), so every post-matmul drain is a single
activation (or pool chain) reading PSUM directly.
"""

import os

import numpy as np
import ml_dtypes

NC = 8      # cores
B = 64      # batch
BC = 8      # images per core

F8NP = ml_dtypes.float8_e4m3fn
EPS = 1e-5

# conv layer tables (layers 3..5): 3x3 convs on 13x13, DoubleRow over ci pairs
#   pr: number of 256-channel ci pairs, n_mi: number of 128-wide co tiles
L3 = dict(name="c3", ci=576, co=1152, pr=3, n_mi=9, pool=False)
L4 = dict(name="c4", ci=1152, co=768, pr=5, n_mi=6, pool=False)
L5 = dict(name="c5", ci=768, co=256, pr=3, n_mi=2, pool=True)

CO2_T = [128, 128, 128, 128, 64]   # conv2 output-channel tiles


# ----------------------------------------------------------------------------
# host-side preprocessing
# ----------------------------------------------------------------------------

def _sgn(x):
    return np.sign(x).astype(np.float32)


def _convx_w_pack(w, n_pr):
    """[CO,CI,3,3] -> [n_mi, 128 p, 9 off, n_pr, 2 j, 128 c] fp8 DoubleRow layout."""
    co, ci, kh, kw = w.shape
    n_mi = co // 128
    sw = _sgn(w)
    out = np.zeros((n_mi, 128, 9, n_pr, 2, 128), np.float32)
    for mi in range(n_mi):
        for pr in range(n_pr):
            for j in range(2):
                ci0 = 256 * pr + 128 * j
                ciw = min(128, ci - ci0)
                if ciw <= 0:
                    continue
                blk = sw[mi * 128:(mi + 1) * 128, ci0:ci0 + ciw].reshape(128, ciw, 9).transpose(1, 2, 0)
                out[mi, :ciw, :, pr, j, :] = blk
    return np.ascontiguousarray(out.reshape(n_mi, 128, 9 * n_pr * 2 * 128)).astype(F8NP)


def _conv2_w_pack(w):
    """[576,192,5,5] -> [5 mi, 128 p, 25 off, 2 j, 128 c] fp8 DoubleRow layout."""
    sw = _sgn(w)  # [576, 192, 5, 5]
    out = np.zeros((5, 128, 25, 2, 128), np.float32)
    for mi, mw in enumerate(CO2_T):
        co0 = mi * 128
        for j in range(2):
            ci0, ciw = j * 128, min(128, 192 - j * 128)
            # [mw, ciw, 5, 5] -> [ciw(p), 25(off), mw(c)]
            blk = sw[co0:co0 + mw, ci0:ci0 + ciw].reshape(mw, ciw, 25).transpose(1, 2, 0)
            out[mi, :ciw, :, j, :mw] = blk
    return np.ascontiguousarray(out.reshape(5, 128, 25 * 2 * 128)).astype(F8NP)


def _im2col_c1(v):
    """[B,3,224,224] -> [384, B, 3025] fp16 (K order (ci,ky,kx), zero-padded)."""
    vp = np.pad(v, ((0, 0), (0, 0), (2, 2), (2, 2)))
    s = vp.strides
    w = np.lib.stride_tricks.as_strided(
        vp, (B, 3, 11, 11, 55, 55), (s[0], s[1], s[2], s[3], s[2] * 4, s[3] * 4))
    w = w.reshape(B, 363, 3025).transpose(1, 0, 2)
    out = np.zeros((384, B, 3025), np.float16)
    out[:363] = w
    return out


def _bn_params(cb, g, bb, m, v):
    """[inv, shf'] with the conv/linear bias folded into the shift."""
    inv = (g.astype(np.float32) / np.sqrt(v.astype(np.float32) + EPS)).astype(np.float32)
    shift = (bb.astype(np.float32) - m.astype(np.float32) * inv
             + cb.astype(np.float32) * inv).astype(np.float32)
    return np.ascontiguousarray(np.stack([inv, shift], 1))


def host_prep(inputs):
    d = inputs
    x = np.asarray(d["x"], np.float32)

    hi = np.rint(x * 256.0).astype(np.float32)
    assert np.abs(hi).max() <= 2040.0
    lo = np.rint((x - hi / 256.0) * float(2 ** 20)).astype(np.float32)
    assert np.abs(lo).max() <= 2048.0
    hi_c = _im2col_c1(hi)
    lo_c = _im2col_c1(lo)

    w1 = np.zeros((384, 192), np.float16)
    w1[:363] = _sgn(d["cw1"]).reshape(192, 363).T
    w1p = np.ascontiguousarray(w1.reshape(3, 128, 192))

    w2p = _conv2_w_pack(d["cw2"])
    w3p = _convx_w_pack(d["cw3"], L3["pr"])
    w4p = _convx_w_pack(d["cw4"], L4["pr"])
    w5p = _convx_w_pack(d["cw5"], L5["pr"])

    p1 = _bn_params(d["cb1"], d["g1"], d["bb1"], d["m1"], d["v1"])
    p2 = _bn_params(d["cb2"], d["g2"], d["bb2"], d["m2"], d["v2"])
    p3 = _bn_params(d["cb3"], d["g3"], d["bb3"], d["m3"], d["v3"])
    p4 = _bn_params(d["cb4"], d["g4"], d["bb4"], d["m4"], d["v4"])
    p5 = _bn_params(d["cb5"], d["g5"], d["bb5"], d["m5"], d["v5"])
    p6 = _bn_params(d["lb1"], d["g6"], d["bb6"], d["m6"], d["v6"])
    p7 = _bn_params(d["lb2"], d["g7"], d["bb7"], d["m7"], d["v7"])
    p8 = _bn_params(d["lb3"], d["g8"], d["bb8"], d["m8"], d["v8"])

    l1t = np.ascontiguousarray(_sgn(d["lw1"]).T)   # [9216, 4096] f32
    l2t = np.ascontiguousarray(_sgn(d["lw2"]).T)   # [4096, 4096]
    l3t = np.ascontiguousarray(_sgn(d["lw3"]).T)   # [4096, 1000]

    def _tile_w(wt, n_kt, cw, dtype):
        """[K, C] -> [C//cw, 128 p, n_kt*cw] (per-tile contiguous DMA)."""
        n_m = wt.shape[1] // cw
        t = wt.reshape(n_kt, 128, wt.shape[1])
        out = np.stack([
            np.ascontiguousarray(t[:, :, m * cw:(m + 1) * cw].transpose(1, 0, 2))
            .reshape(128, n_kt * cw) for m in range(n_m)])
        return np.ascontiguousarray(out).astype(dtype)

    l3p = _tile_w(l3t, 32, 125, F8NP)   # [8, 128, 4000] full (no col shard)

    in_maps = []
    for r in range(NC):
        sl = slice(r * BC, (r + 1) * BC)
        m = dict(
            rh=np.ascontiguousarray(
                hi_c[:, sl].reshape(3, 128, BC, 3025).transpose(2, 1, 0, 3)
            ).reshape(BC, 128, 3 * 3025),
            rl=np.ascontiguousarray(
                lo_c[:, sl].reshape(3, 128, BC, 3025).transpose(2, 1, 0, 3)
            ).reshape(BC, 128, 3 * 3025),
            w1p=w1p, w2p=w2p, w3p=w3p, w4p=w4p, w5p=w5p,
            p1=p1, p2=p2, p3=p3, p4=p4, p5=p5,
            p6=np.ascontiguousarray(p6[r * 512:(r + 1) * 512]),
            p7=np.ascontiguousarray(p7[r * 512:(r + 1) * 512]),
            p8=p8,
            l1t=_tile_w(l1t[:, r * 512:(r + 1) * 512], 72, 128, ml_dtypes.bfloat16),
            l2t=_tile_w(l2t[:, r * 512:(r + 1) * 512], 32, 128, F8NP),
            l3t=l3p,
        )
        in_maps.append(m)
    return in_maps


# ----------------------------------------------------------------------------
# device program
# ----------------------------------------------------------------------------

_CACHE = {}


def build_nc(dump=(), single=False, reps=1):
    import concourse.bass as bass  # noqa: F401
    import concourse.mybir as mybir
    import concourse.tile as tile
    from concourse import bacc
    from concourse.masks import make_identity
    from contextlib import ExitStack

    dt = mybir.dt
    AF = mybir.ActivationFunctionType
    ALU = mybir.AluOpType
    DR = mybir.MatmulPerfMode.DoubleRow
    from concourse.ap import AP as _AP

    def _win(base, extra_off, *dims):
        """Overlapping-window AP: keep base's partition dim, replace free dims."""
        pairs = [[int(p[0]), int(p[1])] for p in base.ap]
        ap = [pairs[0]] + [[s_, n_] for s_, n_ in dims]
        return _AP(base.tensor, base.offset + extra_off, ap)

    nc = bacc.Bacc(num_devices=NC)

    # ---- I/O ----
    rh_t = nc.dram_tensor("rh", [BC, 128, 3 * 3025], dt.float16, kind="ExternalInput")
    rl_t = nc.dram_tensor("rl", [BC, 128, 3 * 3025], dt.float16, kind="ExternalInput")
    w1_t = nc.dram_tensor("w1p", [3, 128, 192], dt.float16, kind="ExternalInput")
    w2_t = nc.dram_tensor("w2p", [5, 128, 25 * 2 * 128], dt.float8e4, kind="ExternalInput")
    wts = {}
    for L in (L3, L4, L5):
        wts[L["name"]] = nc.dram_tensor(
            "w%sp" % L["name"][1], [L["n_mi"], 128, 9 * L["pr"] * 2 * 128],
            dt.float8e4, kind="ExternalInput")
    pt = {}
    for i, c in zip(range(1, 9), (192, 576, 1152, 768, 256, 512, 512, 1000)):
        pt[i] = nc.dram_tensor("p%d" % i, [c, 2], dt.float32, kind="ExternalInput")
    l1_t = nc.dram_tensor("l1t", [4, 128, 72 * 128], dt.bfloat16, kind="ExternalInput")
    l2_t = nc.dram_tensor("l2t", [4, 128, 32 * 128], dt.float8e4, kind="ExternalInput")
    l3_t = nc.dram_tensor("l3t", [8, 128, 32 * 125], dt.float8e4, kind="ExternalInput")
    out_t = nc.dram_tensor("out", [64, 1000], dt.float32, kind="ExternalOutput")

    RG = [list(range(NC))]

    def allgather(blk, G, rows):
        if single:
            nc.gpsimd.dma_start(G[0:rows], blk[:])
        else:
            nc.gpsimd.collective_compute("AllGather", mybir.AluOpType.bypass, replica_groups=RG,
                                         ins=[blk[:].opt()], outs=[G[:].opt()])

    with tile.TileContext(nc) as tc:
        stacks = {}

        cur_pfx = [""]

        def open_pool(key, **kw):
            key = cur_pfx[0] + key
            s = ExitStack()
            p = s.enter_context(tc.tile_pool(name=key, **kw))
            stacks[key] = s
            return p

        def close_pool(key):
            stacks.pop(cur_pfx[0] + key).close()

        for _rep in range(reps):
            _pfx = "r%d_" % _rep if reps > 1 else ""
            cur_pfx[0] = _pfx
            blk1 = nc.dram_tensor(_pfx + "blk1", [BC, 9216], dt.bfloat16)
            G1 = nc.dram_tensor(_pfx + "G1", [64, 9216], dt.bfloat16,
                                addr_space="Shared")
            blk2 = nc.dram_tensor(_pfx + "blk2", [512, 64], dt.float8e4)
            G2 = nc.dram_tensor(_pfx + "G2", [4096, 64], dt.float8e4,
                                addr_space="Shared")
            blk3 = nc.dram_tensor(_pfx + "blk3", [512, 64], dt.float8e4)
            G3 = nc.dram_tensor(_pfx + "G3", [4096, 64], dt.float8e4,
                                addr_space="Shared")

            # ------------------------------------------------------------------
            # stage 1: conv1 (fp16 hi/lo) -> pool -> bn -> sign -> a1 (fp8 DR)
            # ------------------------------------------------------------------
            ppar = open_pool("par", bufs=1)
            pa1 = open_pool("a1", bufs=1)
            pr1 = open_pool("r1", bufs=2)
            pw1 = open_pool("w1", bufs=1)
            pc1 = open_pool("c1", bufs=2)
            pps1 = open_pool("ps1", bufs=3, space="PSUM")

            w1sb = pw1.tile([128, 3, 192], dt.float16)
            nc.sync.dma_start(w1sb[:], w1_t[:].rearrange("k p c -> p k c"))

            # conv2 input, DoubleRow layout: [p, j(ci half), b, y, x]
            a1dr = pa1.tile([128, 2, BC, 31, 32], dt.float8e4, name="a1dr")
            _ms_engs = [nc.gpsimd, nc.vector]
            for b in range(BC):
                _ms_engs[b % 2].memset(a1dr[:, :, b], 0.0)
            par1s = []
            for m, (m0, mw) in enumerate(((0, 128), (128, 64))):
                par1 = ppar.tile([128, 2], dt.float32, tag="par1_%d" % m)
                nc.sync.dma_start(par1[0:mw], pt[1][m0:m0 + mw, :])
                par1s.append(par1)

            NT1 = [(i * 512, min(512, 3025 - i * 512)) for i in range(6)]
            for b in range(BC):
                rht = pr1.tile([128, 3, 3025], dt.float16, tag="rh")
                nc.gpsimd.dma_start(rht[:].rearrange("p kt n -> p (kt n)"), rh_t[b])
                rlt = pr1.tile([128, 3, 3025], dt.float16, tag="rl")
                nc.scalar.dma_start(rlt[:].rearrange("p kt n -> p (kt n)"), rl_t[b])
                c1i = [pc1.tile([128, 3025], dt.float32, tag="c1_%d" % m, name="c1_%d" % m) for m in range(2)]
                for m, (m0, mw) in enumerate(((0, 128), (128, 64))):
                    par1 = par1s[m]
                    for n0, nn in NT1:
                        ph = pps1.tile([128, 512], dt.float32, tag="ph")
                        pl = pps1.tile([128, 512], dt.float32, tag="pl")
                        for kt in range(3):
                            nc.tensor.matmul(ph[0:mw, 0:nn], w1sb[:, kt, m0:m0 + mw],
                                             rht[:, kt, n0:n0 + nn], start=kt == 0, stop=kt == 2)
                        for kt in range(3):
                            nc.tensor.matmul(pl[0:mw, 0:nn], w1sb[:, kt, m0:m0 + mw],
                                             rlt[:, kt, n0:n0 + nn], start=kt == 0, stop=kt == 2)
                        tlo = pc1.tile([128, 512], dt.float32, tag="tlo")
                        nc.scalar.activation(tlo[0:mw, 0:nn], pl[0:mw, 0:nn],
                                             AF.Copy, scale=float(2 ** -20))
                        nc.vector.scalar_tensor_tensor(c1i[m][0:mw, n0:n0 + nn], ph[0:mw, 0:nn],
                                                       float(2 ** -8), tlo[0:mw, 0:nn], ALU.mult, ALU.add)
                    # maxpool 55->27 (rows then cols)
                    v = c1i[m][0:mw].rearrange("p (y x) -> p y x", x=55)
                    ty = pc1.tile([128, 27, 55], dt.float32, tag="ty1")
                    nc.vector.tensor_tensor(ty[0:mw], v[:, 0:53:2, :], v[:, 1:54:2, :], ALU.max)
                    nc.vector.tensor_tensor(ty[0:mw], ty[0:mw], v[:, 2:55:2, :], ALU.max)
                    pld = pc1.tile([128, 27, 27], dt.float32, tag="pl1")
                    nc.vector.tensor_tensor(pld[0:mw], ty[0:mw, :, 0:53:2], ty[0:mw, :, 1:54:2], ALU.max)
                    nc.vector.tensor_tensor(pld[0:mw], pld[0:mw], ty[0:mw, :, 2:55:2], ALU.max)
                    nc.scalar.activation(a1dr[0:mw, m, b, 2:29, 2:29], pld[0:mw], AF.Sign,
                                         bias=par1[0:mw, 1:2], scale=par1[0:mw, 0:1])
            close_pool("ps1"); close_pool("c1"); close_pool("w1"); close_pool("r1")

            # ------------------------------------------------------------------
            # conv2: fp8 DoubleRow (K=256) -> a2x pair tiles [p, j, b2, 15, 32]
            # (2 images x-packed per plane: img s at cols 16s+1..16s+13)
            # ------------------------------------------------------------------
            pa2 = open_pool("a2", bufs=1)
            a2x = [pa2.tile([128, 2, 4, 15, 32], dt.float8e4, tag="a2_%d" % i, name="a2_%d" % i)
                   for i in range(L3["pr"])]
            for i, t in enumerate(a2x):
                for b2 in range(4):
                    _ms_engs[(i * 4 + b2) % 2].memset(t[:, :, b2], 0.0)
            pw2 = open_pool("w2", bufs=2)
            pc2 = open_pool("cc2", bufs=4)
            pps2 = open_pool("ps2", bufs=1, space="PSUM")

            co0 = 0
            for mi, mw in enumerate(CO2_T):
                wsb = pw2.tile([128, 25, 2, 128], dt.float8e4, tag="w2")
                nc.sync.dma_start(wsb[:].rearrange("p k j c -> p (k j c)"), w2_t[mi])
                part = pw2.tile([128, 2], dt.float32, tag="par2")
                nc.sync.dma_start(part[0:mw], pt[2][co0:co0 + mw, :])
                inv = part[0:mw, 0:1]
                shf = part[0:mw, 1:2]
                oc = co0 // 128
                op = co0 % 128
                for g in range(2):
                    psA = [pps2.tile([128, 18, 27], dt.float32, tag="psA%d" % i, name="psA%d" % i) for i in range(4)]
                    psB = [pps2.tile([128, 9, 27], dt.float32, tag="psB%d" % i, name="psB%d" % i) for i in range(4)]
                    for off in range(25):
                        ky, kx = off // 5, off % 5
                        for i in range(4):
                            b = 4 * g + i
                            nc.tensor.matmul(psA[i][0:mw], wsb[:, off, :, 0:mw],
                                             a1dr[:, :, b, ky:ky + 18, kx:kx + 27],
                                             start=off == 0, stop=off == 24, perf_mode=DR)
                            nc.tensor.matmul(psB[i][0:mw], wsb[:, off, :, 0:mw],
                                             a1dr[:, :, b, 18 + ky:27 + ky, kx:kx + 27],
                                             start=off == 0, stop=off == 24, perf_mode=DR)
                    for i in range(4):
                        b = 4 * g + i
                        c2 = pc2.tile([128, 27, 27], dt.float32, tag="img2")
                        nc.scalar.activation(c2[0:mw, 0:18], psA[i][0:mw], AF.Copy)
                        nc.scalar.activation(c2[0:mw, 18:27], psB[i][0:mw], AF.Copy)
                        ty = pc2.tile([128, 13, 27], dt.float32, tag="ty2")
                        nc.vector.tensor_tensor(ty[0:mw], c2[0:mw, 0:25:2], c2[0:mw, 1:26:2], ALU.max)
                        nc.vector.tensor_tensor(ty[0:mw], ty[0:mw], c2[0:mw, 2:27:2], ALU.max)
                        pld = pc2.tile([128, 13, 13], dt.float32, tag="pool2")
                        nc.vector.tensor_tensor(pld[0:mw], ty[0:mw, :, 0:25:2], ty[0:mw, :, 1:26:2], ALU.max)
                        nc.vector.tensor_tensor(pld[0:mw], pld[0:mw], ty[0:mw, :, 2:27:2], ALU.max)
                        x0 = 16 * (b % 2) + 1
                        nc.scalar.activation(a2x[mi // 2][0:mw, mi % 2, b // 2, 1:14, x0:x0 + 13],
                                             pld[0:mw], AF.Sign, bias=shf, scale=inv)
                co0 += mw
            close_pool("ps2"); close_pool("cc2"); close_pool("w2")

            # ------------------------------------------------------------------
            # conv layers 3..5: fp8 DoubleRow over ci pairs, x-packed image
            # pairs (rhs [128, 2, 13, 29], N=377; out img s at psum cols 16s..)
            # ------------------------------------------------------------------
            def conv_layer(L, idx, ain, out_pr):
                n_pr, n_mi = L["pr"], L["n_mi"]
                name = L["name"]
                KT = 9 * n_pr
                if name == "c5":
                    pa = open_pool("a5", bufs=1)
                    aout = [pa.tile([128, BC, 36], dt.bfloat16, tag="a5_%d" % i, name="a5_%d" % i) for i in range(2)]
                else:
                    pa = open_pool("a" + str(idx), bufs=1)
                    aout = [pa.tile([128, 2, 4, 15, 32], dt.float8e4, tag="a%d_%d" % (idx, i), name="a%d_%d" % (idx, i))
                            for i in range(out_pr)]
                    for i, t in enumerate(aout):
                        for b2 in range(4):
                            _ms_engs[(i * 4 + b2) % 2].memset(t[:, :, b2], 0.0)
                pw = open_pool("w" + name, bufs=2)
                pc = open_pool("cc" + name, bufs=4)
                pps = open_pool("ps" + name, bufs=6, space="PSUM")

                for mi in range(n_mi):
                    wsb = pw.tile([128, 9, n_pr, 2, 128], dt.float8e4, tag="w" + name)
                    nc.sync.dma_start(wsb[:].rearrange("p k r j c -> p (k r j c)"), wts[name][mi])
                    part = pw.tile([128, 2], dt.float32, tag="par" + name)
                    nc.sync.dma_start(part[:], pt[idx][mi * 128:(mi + 1) * 128, :])
                    inv = part[:, 0:1]
                    shf = part[:, 1:2]
                    for b2 in range(4):
                        ps = pps.tile([128, 13, 29], dt.float32, tag="ps" + name)
                        k = 0
                        for off in range(9):
                            ky, kx = off // 3, off % 3
                            for pr in range(n_pr):
                                nc.tensor.matmul(
                                    ps[:], wsb[:, off, pr, :, :],
                                    ain[pr][:, :, b2, ky:ky + 13, kx:kx + 29],
                                    start=k == 0, stop=k == KT - 1, perf_mode=DR)
                                k += 1
                        if name == "c5":
                            y5 = pc.tile([128, 13, 29], dt.float32, tag="img" + name)
                            nc.scalar.activation(y5[:], ps[:], AF.Copy)
                            for s in range(2):
                                b = 2 * b2 + s
                                x0 = 16 * s
                                ty = pc.tile([128, 6, 13], dt.float32, tag="ty" + name)
                                nc.vector.tensor_tensor(ty[:], y5[:, 0:11:2, x0:x0 + 13],
                                                        y5[:, 1:12:2, x0:x0 + 13], ALU.max)
                                nc.vector.tensor_tensor(ty[:], ty[:], y5[:, 2:13:2, x0:x0 + 13], ALU.max)
                                pld = pc.tile([128, 6, 6], dt.float32, tag="pool" + name)
                                nc.vector.tensor_tensor(pld[:], ty[:, :, 0:11:2],
                                                        ty[:, :, 1:12:2], ALU.max)
                                nc.vector.tensor_tensor(pld[:], pld[:], ty[:, :, 2:13:2], ALU.max)
                                nc.scalar.activation(
                                    aout[mi][:, b].rearrange("p (y x) -> p y x", x=6),
                                    pld[:], AF.Sign, bias=shf, scale=inv)
                        else:
                            for s in range(2):
                                x0 = 16 * s
                                nc.scalar.activation(
                                    aout[mi // 2][:, mi % 2, b2, 1:14, x0 + 1:x0 + 14],
                                    ps[:, :, x0:x0 + 13], AF.Sign, bias=shf, scale=inv)
                close_pool("ps" + name); close_pool("cc" + name); close_pool("w" + name)
                return aout

            a3x = conv_layer(L3, 3, a2x, L4["pr"])
            a4x = conv_layer(L4, 4, a3x, L5["pr"])
            a5t = conv_layer(L5, 5, a4x, 0)

            # ------------------------------------------------------------------
            # FC layers (tensor parallel, fp8 where no transpose-DMA is needed;
            # FC3 computed in full on every core so no logit collective)
            # ------------------------------------------------------------------
            pfc = open_pool("fc", bufs=2)
            pfw1 = open_pool("fw1", bufs=3)
            pfw2 = open_pool("fw2", bufs=4)
            pfw3 = open_pool("fw3", bufs=5)
            pfwp = open_pool("fwp", bufs=2)
            ppsf = open_pool("psf", bufs=4, space="PSUM")

            # a5 -> blk1 [8, 9216] (b-major rows)
            b1v = blk1[:].rearrange("b (c hw) -> c b hw", hw=36)
            for ch in range(2):
                nc.gpsimd.dma_start(b1v[ch * 128:(ch + 1) * 128], a5t[ch][:])
            allgather(blk1, G1, BC)

            # FC1: rhs via transpose DMAs
            r1t = pfc.tile([128, 72, 64], dt.bfloat16, tag="r1")
            for kc in range(72):
                nc.sync.dma_start(r1t[:, kc, :], G1[0:64, kc * 128:(kc + 1) * 128], transpose=True)
            for m in range(4):
                wsb = pfw1.tile([128, 72, 128], dt.bfloat16, tag="l1w")
                nc.sync.dma_start(wsb[:].rearrange("p k c -> p (k c)"), l1_t[m])
                par6 = pfwp.tile([128, 2], dt.float32, tag="par6")
                nc.sync.dma_start(par6[:], pt[6][m * 128:(m + 1) * 128, :])
                ps = ppsf.tile([128, 64], dt.float32, tag="psf")
                for kc in range(72):
                    nc.tensor.matmul(ps[:], wsb[:, kc, :], r1t[:, kc, :], start=kc == 0, stop=kc == 71)
                a6 = pfc.tile([128, 64], dt.float8e4, tag="a6")
                nc.scalar.activation(a6[:], ps[:], AF.Sign,
                                     bias=par6[:, 1:2], scale=par6[:, 0:1])
                nc.gpsimd.dma_start(blk2[m * 128:(m + 1) * 128, :], a6[:])
            allgather(blk2, G2, 512)

            # FC2 (fp8)
            r2t = pfc.tile([128, 32, 64], dt.float8e4, tag="r2")
            nc.sync.dma_start(r2t[:], G2[:].rearrange("(kt p) b -> p kt b", p=128))
            for m in range(4):
                wsb = pfw2.tile([128, 32, 128], dt.float8e4, tag="l2w")
                nc.sync.dma_start(wsb[:].rearrange("p k c -> p (k c)"), l2_t[m])
                par7 = pfwp.tile([128, 2], dt.float32, tag="par7")
                nc.sync.dma_start(par7[:], pt[7][m * 128:(m + 1) * 128, :])
                ps = ppsf.tile([128, 64], dt.float32, tag="psf")
                for kc in range(32):
                    nc.tensor.matmul(ps[:], wsb[:, kc, :], r2t[:, kc, :], start=kc == 0, stop=kc == 31)
                a7 = pfc.tile([128, 64], dt.float8e4, tag="a6")
                nc.scalar.activation(a7[:], ps[:], AF.Sign,
                                     bias=par7[:, 1:2], scale=par7[:, 0:1])
                nc.gpsimd.dma_start(blk3[m * 128:(m + 1) * 128, :], a7[:])
            allgather(blk3, G3, 512)

            # FC3 + bn8, all 1000 columns on every core (fp8), transpose into lg
            r3t = pfc.tile([128, 32, 64], dt.float8e4, tag="r3")
            nc.sync.dma_start(r3t[:], G3[:].rearrange("(kt p) b -> p kt b", p=128))
            idt = ppar.tile([128, 128], dt.float32, tag="ident")
            make_identity(nc, idt[:])
            lg = pfc.tile([64, 1000], dt.float32, tag="lg")
            for m in range(8):
                w3sb = pfw3.tile([128, 32, 125], dt.float8e4, tag="l3w")
                nc.sync.dma_start(w3sb[:].rearrange("p k c -> p (k c)"), l3_t[m])
                par8 = pfwp.tile([125, 2], dt.float32, tag="par8")
                nc.sync.dma_start(par8[:], pt[8][m * 125:(m + 1) * 125, :])
                ps = ppsf.tile([128, 64], dt.float32, tag="psf")
                for kc in range(32):
                    nc.tensor.matmul(ps[0:125, :], w3sb[:, kc, :], r3t[:, kc, :], start=kc == 0, stop=kc == 31)
                z8 = pfc.tile([125, 64], dt.float32, tag="z8")
                nc.vector.tensor_scalar(z8[:], ps[0:125, :], par8[:, 0:1], par8[:, 1:2], ALU.mult, ALU.add)
                tp = ppsf.tile([64, 128], dt.float32, tag="tp")
                nc.tensor.transpose(tp[0:64, 0:125], z8[:], idt[0:125, 0:125])
                nc.vector.tensor_copy(lg[:, m * 125:(m + 1) * 125], tp[0:64, 0:125])
            mx = pfc.tile([64, 1], dt.float32, tag="mx")
            nc.vector.reduce_max(mx[:], lg[:], axis=mybir.AxisListType.X)
            sh = pfc.tile([64, 1000], dt.float32, tag="sh")
            nc.vector.tensor_scalar(sh[:], lg[:], mx[:], None, ALU.subtract)
            ex = pfc.tile([64, 1000], dt.float32, tag="ex")
            nc.scalar.activation(ex[:], sh[:], AF.Exp)
            sm = pfc.tile([64, 1], dt.float32, tag="sm")
            nc.vector.reduce_sum(sm[:], ex[:], axis=mybir.AxisListType.X)
            ls = pfc.tile([64, 1], dt.float32, tag="ls")
            nc.scalar.activation(ls[:], sm[:], AF.Ln)
            osb = pfc.tile([64, 1000], dt.float32, tag="osb")
            nc.vector.tensor_scalar(osb[:], sh[:], ls[:], None, ALU.subtract)
            nc.sync.dma_start(out_t[:], osb[:])

            for k in reversed(list(stacks)):
                stacks.pop(k).close()

    nc.finalize()
    return nc


def get_nc(dump=()):
    key = tuple(sorted(dump))
    if key not in _CACHE:
        _CACHE[key] = build_nc(dump)
    return _CACHE[key]


def kernel(**inputs):
    from concourse.bass_utils import run_bass_kernel_spmd

    nc = get_nc(())
    in_maps = host_prep(inputs)
    res = run_bass_kernel_spmd(nc, in_maps, core_ids=list(range(NC)))
    out = np.asarray(res.results[0]["out"], np.float32)
    return out

